# revision 1
# baseline (speedup 1.0000x reference)
import numpy as np

# nn_GatedGATLayer: B=2, n=512, H=64. Shard the destination-node axis i
# across the 8 NeuronCores (softmax/aggregation are over j within each i
# row, so no cross-device reduction); replicate X and the HxH weights.
H = 64
EPS = 1e-5
B, N = 2, 512
NCORES = 8


def _np_kernel(X, E, W3, W4, W5, W6, gn_g, gn_b, ge_g, ge_b):
    X = np.asarray(X, np.float32)
    E = np.asarray(E, np.float32)
    Xi = X @ W3.T
    E_hat = E @ W4.T
    xi_s = Xi.sum(-1)
    logits = xi_s[:, :, None] + xi_s[:, None, :] + E_hat.sum(-1)
    m = logits.max(-1, keepdims=True)
    p = np.exp(logits - m)
    att = (p / p.sum(-1, keepdims=True))[..., None]
    E_next = E_hat * att
    Xj = X @ W5.T
    agg = np.einsum('bijh,bjh->bih', E_next, Xj)
    X_res = X @ W6.T
    pre = np.maximum(agg + X_res, 0.0)
    mu = pre.mean(-1, keepdims=True)
    v = np.square(pre - mu).mean(-1, keepdims=True)
    X_next = (pre - mu) / np.sqrt(v + EPS) * gn_g + gn_b + X
    mu2 = E_next.mean(-1, keepdims=True)
    v2 = np.square(E_next - mu2).mean(-1, keepdims=True)
    E_out = (E_next - mu2) / np.sqrt(v2 + EPS) * ge_g + ge_b
    return X_next.astype(np.float32), E_out.astype(np.float32)


def _jax_kernel(X, E, W3, W4, W5, W6, gn_g, gn_b, ge_g, ge_b):
    import jax
    import jax.numpy as jnp
    from jax.sharding import Mesh, NamedSharding, PartitionSpec as P

    devs = jax.devices()
    if len(devs) < NCORES:
        raise RuntimeError("need 8 cores")
    mesh = Mesh(np.asarray(devs[:NCORES]), ('i',))
    rep = NamedSharding(mesh, P())
    sh_E = NamedSharding(mesh, P(None, 'i', None, None))
    sh_X = NamedSharding(mesh, P(None, 'i', None))

    def f(X, E, W3, W4, W5, W6, gn_g, gn_b, ge_g, ge_b):
        Xi = jnp.einsum('bnh,oh->bno', X, W3)
        E_hat = jnp.einsum('bijh,oh->bijo', E, W4)
        xi_s = jnp.sum(Xi, axis=-1)
        logits = xi_s[:, :, None] + xi_s[:, None, :] + jnp.sum(E_hat, axis=-1)
        att = jax.nn.softmax(logits, axis=-1)[..., None]
        E_next = E_hat * att
        Xj = jnp.einsum('bnh,oh->bno', X, W5)
        agg = jnp.einsum('bijh,bjh->bih', E_next, Xj)
        X_res = jnp.einsum('bnh,oh->bno', X, W6)
        pre = jax.nn.relu(agg + X_res)
        mu = jnp.mean(pre, axis=-1, keepdims=True)
        v = jnp.mean(jnp.square(pre - mu), axis=-1, keepdims=True)
        X_next = (pre - mu) * jax.lax.rsqrt(v + EPS) * gn_g + gn_b + X
        mu2 = jnp.mean(E_next, axis=-1, keepdims=True)
        v2 = jnp.mean(jnp.square(E_next - mu2), axis=-1, keepdims=True)
        E_out = (E_next - mu2) * jax.lax.rsqrt(v2 + EPS) * ge_g + ge_b
        return X_next, E_out

    jf = jax.jit(
        f,
        in_shardings=(rep, sh_E, rep, rep, rep, rep, rep, rep, rep, rep),
        out_shardings=(sh_X, sh_E),
    )
    Xn, Eo = jf(
        jnp.asarray(X, jnp.float32), jnp.asarray(E, jnp.float32),
        jnp.asarray(W3, jnp.float32), jnp.asarray(W4, jnp.float32),
        jnp.asarray(W5, jnp.float32), jnp.asarray(W6, jnp.float32),
        jnp.asarray(gn_g, jnp.float32), jnp.asarray(gn_b, jnp.float32),
        jnp.asarray(ge_g, jnp.float32), jnp.asarray(ge_b, jnp.float32),
    )
    return np.asarray(Xn), np.asarray(Eo)


def kernel(X, E, W3, W4, W5, W6, gn_g, gn_b, ge_g, ge_b):
    try:
        return _jax_kernel(X, E, W3, W4, W5, W6, gn_g, gn_b, ge_g, ge_b)
    except Exception:
        return _np_kernel(X, E, W3, W4, W5, W6, gn_g, gn_b, ge_g, ge_b)


# revision 2
# speedup vs baseline: 1.2337x; 1.2337x over previous
import numpy as np

# nn_GatedGATLayer: B=2, n=512, H=64. Shard the destination-node axis i
# across the 8 NeuronCores (softmax/aggregation are over j within each i
# row, so no cross-device reduction); replicate X and the HxH weights.
H = 64
EPS = 1e-5
B, N = 2, 512
NCORES = 8


def _np_kernel(X, E, W3, W4, W5, W6, gn_g, gn_b, ge_g, ge_b):
    X = np.asarray(X, np.float32)
    E = np.asarray(E, np.float32)
    Xi = X @ W3.T
    E_hat = E @ W4.T
    xi_s = Xi.sum(-1)
    logits = xi_s[:, :, None] + xi_s[:, None, :] + E_hat.sum(-1)
    m = logits.max(-1, keepdims=True)
    p = np.exp(logits - m)
    att = (p / p.sum(-1, keepdims=True))[..., None]
    E_next = E_hat * att
    Xj = X @ W5.T
    agg = np.einsum('bijh,bjh->bih', E_next, Xj)
    X_res = X @ W6.T
    pre = np.maximum(agg + X_res, 0.0)
    mu = pre.mean(-1, keepdims=True)
    v = np.square(pre - mu).mean(-1, keepdims=True)
    X_next = (pre - mu) / np.sqrt(v + EPS) * gn_g + gn_b + X
    mu2 = E_next.mean(-1, keepdims=True)
    v2 = np.square(E_next - mu2).mean(-1, keepdims=True)
    E_out = (E_next - mu2) / np.sqrt(v2 + EPS) * ge_g + ge_b
    return X_next.astype(np.float32), E_out.astype(np.float32)


def _jax_kernel(X, E, W3, W4, W5, W6, gn_g, gn_b, ge_g, ge_b):
    import jax
    import jax.numpy as jnp
    from jax.sharding import Mesh, NamedSharding, PartitionSpec as P

    devs = jax.devices()
    if len(devs) < NCORES:
        raise RuntimeError("need 8 cores")
    mesh = Mesh(np.asarray(devs[:NCORES]), ('i',))
    rep = NamedSharding(mesh, P())
    sh_E = NamedSharding(mesh, P(None, 'i', None, None))
    sh_X = NamedSharding(mesh, P(None, 'i', None))

    def f(X, E, W3, W4, W5, W6, gn_g, gn_b, ge_g, ge_b):
        Xi = jnp.einsum('bnh,oh->bno', X, W3)
        E_hat = jnp.einsum('bijh,oh->bijo', E, W4)
        xi_s = jnp.sum(Xi, axis=-1)
        logits = xi_s[:, :, None] + xi_s[:, None, :] + jnp.sum(E_hat, axis=-1)
        att = jax.nn.softmax(logits, axis=-1)[..., None]
        E_next = E_hat * att
        Xj = jnp.einsum('bnh,oh->bno', X, W5)
        agg = jnp.einsum('bijh,bjh->bih', E_next, Xj)
        X_res = jnp.einsum('bnh,oh->bno', X, W6)
        pre = jax.nn.relu(agg + X_res)
        mu = jnp.mean(pre, axis=-1, keepdims=True)
        v = jnp.mean(jnp.square(pre - mu), axis=-1, keepdims=True)
        X_next = (pre - mu) * jax.lax.rsqrt(v + EPS) * gn_g + gn_b + X
        mu2 = jnp.mean(E_next, axis=-1, keepdims=True)
        v2 = jnp.mean(jnp.square(E_next - mu2), axis=-1, keepdims=True)
        E_out = (E_next - mu2) * jax.lax.rsqrt(v2 + EPS) * ge_g + ge_b
        return X_next, E_out

    jf = jax.jit(
        f,
        in_shardings=(rep, sh_E, rep, rep, rep, rep, rep, rep, rep, rep),
        out_shardings=(sh_X, sh_E),
    )
    args = [np.ascontiguousarray(np.asarray(a, np.float32))
            for a in (X, E, W3, W4, W5, W6, gn_g, gn_b, ge_g, ge_b)]
    Xn, Eo = jf(*args)
    return np.asarray(Xn), np.asarray(Eo)


def kernel(X, E, W3, W4, W5, W6, gn_g, gn_b, ge_g, ge_b):
    try:
        return _jax_kernel(X, E, W3, W4, W5, W6, gn_g, gn_b, ge_g, ge_b)
    except Exception:
        return _np_kernel(X, E, W3, W4, W5, W6, gn_g, gn_b, ge_g, ge_b)
